# revision 2
# baseline (speedup 1.0000x reference)
"""FCOS box decode + class-aware NMS (greedy, IoU>0.5) on 8 Trainium2 cores.

Strategy
--------
The reference does: decode boxes -> class-offset trick -> sort by score ->
dense 8192x8192 pairwise IoU -> sequential greedy suppression sweep.

Key observation: the IoU>0.5 "suppression" predicate is symmetric and boxes
are small (<=256px) relative to the offset coordinate space, so if boxes are
sorted by their (class-offset) left edge x1, a box can only overlap a short
contiguous run of the boxes that follow it: every pair outside the run has
x1[j] > x2[i], which makes the reference's clip(xx2-xx1,0) exactly 0 and
hence IoU exactly 0 (this is a comparison of stored f32 values - no rounding
assumptions).  The device therefore computes only banded predicate tiles
(64 chunks x [128, W] with W determined from the data, ~256), packs the
0/1 predicate into bits, and ships them back.  The suppression graph is
extremely sparse (~1.7k edges out of 33M candidate pairs), so the greedy
sweep itself is O(edges) host glue, as is the argsort.

Sharding: the 64 row-chunks are split contiguously across the 8 cores
(8 chunks each, perfectly balanced since W is uniform).  Every core also
redundantly decodes the boxes output (cheap); core 0's copy is returned.

The device predicate replicates the reference's arithmetic exactly except
the final divide: instead of iou = inter/union > 0.5 it tests
inter > 0.5*union (0.5*union is exact in f32), which can only disagree when
the exact quotient lies within half an ulp of 0.5 - probability ~1e-3 over
the whole dataset.
"""

import numpy as np

N = 8192
CH = 128                 # rows per chunk (= SBUF partitions)
NCHUNK = N // CH         # 64
NCORES = 8
CPC = NCHUNK // NCORES   # chunks per core

_PROGRAM_CACHE = {}


def _build_program(W, stride_f):
    import concourse.bass as bass
    import concourse.tile as tile
    from concourse import bacc, mybir

    f32 = mybir.dt.float32
    u8 = mybir.dt.uint8
    Alu = mybir.AluOpType
    Lc = CPC * CH + W    # columns a core needs (its 8 chunks + trailing window)

    nc = bacc.Bacc(
        "TRN2", target_bir_lowering=False, debug=False, num_devices=NCORES
    )

    deltas_t = nc.dram_tensor("deltas", [N, 4], f32, kind="ExternalInput").ap()
    locs_t = nc.dram_tensor("locations", [N, 2], f32, kind="ExternalInput").ap()
    cols_t = nc.dram_tensor("cols", [128, 5, Lc], f32, kind="ExternalInput").ap()
    rows_t = nc.dram_tensor("rows", [128, CPC, 5], f32, kind="ExternalInput").ap()
    wbits_t = nc.dram_tensor("wbits", [128, W], f32, kind="ExternalInput").ap()
    boxes_o = nc.dram_tensor("boxes", [N, 4], f32, kind="ExternalOutput").ap()
    packed_o = nc.dram_tensor(
        "packed", [CPC, 128, W // 8], u8, kind="ExternalOutput"
    ).ap()

    with tile.TileContext(nc) as tc:
        with (
            tc.tile_pool(name="const", bufs=1) as cp,
            tc.tile_pool(name="work", bufs=2) as wp,
        ):
            colsb = cp.tile([128, 5, Lc], f32, tag="colsb")
            nc.sync.dma_start(colsb[:], cols_t[:])
            rowsb = cp.tile([128, CPC, 5], f32, tag="rowsb")
            nc.sync.dma_start(rowsb[:], rows_t[:])
            wbitsb = cp.tile([128, W], f32, tag="wbitsb")
            nc.sync.dma_start(wbitsb[:], wbits_t[:])

            # ---- box decode (original order), all cores redundantly ----
            dl = wp.tile([128, 64, 4], f32, tag="dl")
            nc.sync.dma_start(dl[:], deltas_t.rearrange("(p f) c -> p f c", p=128))
            ll = wp.tile([128, 64, 2], f32, tag="ll")
            nc.sync.dma_start(ll[:], locs_t.rearrange("(p f) c -> p f c", p=128))
            dr = wp.tile([128, 64, 4], f32, tag="dr")
            nc.vector.tensor_scalar_max(dr[:], dl[:], 0.0)
            bx = wp.tile([128, 64, 4], f32, tag="bx")
            for c, (sgn, lc) in enumerate([(-1, 0), (-1, 1), (1, 0), (1, 1)]):
                nc.vector.scalar_tensor_tensor(
                    bx[:, :, c], dr[:, :, c], sgn * stride_f, ll[:, :, lc],
                    op0=Alu.mult, op1=Alu.add,
                )
            nc.sync.dma_start(boxes_o.rearrange("(p f) c -> p f c", p=128), bx[:])

            # ---- banded suppression-predicate tiles ----
            for t in range(CPC):
                def cs(q, t=t):
                    return colsb[:, q, t * CH : t * CH + W]

                def rs(q, t=t):
                    return rowsb[:, t, q : q + 1]

                xx1 = wp.tile([128, W], f32, tag="xx1")
                nc.vector.tensor_scalar_max(xx1[:], cs(0), rs(0))
                xx2 = wp.tile([128, W], f32, tag="xx2")
                nc.vector.tensor_scalar_min(xx2[:], cs(2), rs(2))
                w_ = wp.tile([128, W], f32, tag="w_")
                nc.vector.tensor_sub(w_[:], xx2[:], xx1[:])
                wr = wp.tile([128, W], f32, tag="wr")
                nc.vector.tensor_scalar_max(wr[:], w_[:], 0.0)

                yy1 = wp.tile([128, W], f32, tag="yy1")
                nc.vector.tensor_scalar_max(yy1[:], cs(1), rs(1))
                yy2 = wp.tile([128, W], f32, tag="yy2")
                nc.vector.tensor_scalar_min(yy2[:], cs(3), rs(3))
                h_ = wp.tile([128, W], f32, tag="h_")
                nc.vector.tensor_sub(h_[:], yy2[:], yy1[:])
                hr = wp.tile([128, W], f32, tag="hr")
                nc.vector.tensor_scalar_max(hr[:], h_[:], 0.0)

                inter = wp.tile([128, W], f32, tag="inter")
                nc.vector.tensor_mul(inter[:], wr[:], hr[:])
                asum = wp.tile([128, W], f32, tag="asum")
                nc.vector.tensor_scalar_add(asum[:], cs(4), rs(4))
                u_ = wp.tile([128, W], f32, tag="u_")
                nc.vector.tensor_sub(u_[:], asum[:], inter[:])
                # pred = (0.5*u < inter); 0.5*u is exact in f32
                pred = wp.tile([128, W], f32, tag="pred")
                nc.vector.scalar_tensor_tensor(
                    pred[:], u_[:], 0.5, inter[:], op0=Alu.mult, op1=Alu.is_lt
                )
                pm = wp.tile([128, W], f32, tag="pm")
                nc.vector.tensor_mul(pm[:], pred[:], wbitsb[:])
                pkf = wp.tile([128, W // 8], f32, tag="pkf")
                nc.vector.tensor_reduce(
                    pkf[:], pm[:].rearrange("p (g e) -> p g e", e=8),
                    axis=mybir.AxisListType.X, op=Alu.add,
                )
                pk8 = wp.tile([128, W // 8], u8, tag="pk8")
                nc.vector.tensor_copy(pk8[:], pkf[:])
                nc.sync.dma_start(packed_o[t], pk8[:])

    nc.compile()
    return nc


def kernel(deltas, locations, scores, class_ids, stride):
    deltas = np.asarray(deltas, np.float32)
    locations = np.asarray(locations, np.float32)
    scores = np.asarray(scores, np.float32)
    class_ids = np.asarray(class_ids, np.int32)
    stride_f = float(np.asarray(stride))
    n = deltas.shape[0]
    assert n == N

    # ---- host: decode (for sort/band prep only; boxes output comes from
    # the device), class offsets, x1-sort, window size ----
    dd = np.clip(deltas, 0, None)
    xc, yc = locations[:, 0], locations[:, 1]
    s8 = np.float32(stride_f)
    bx = np.stack(
        [xc - dd[:, 0] * s8, yc - dd[:, 1] * s8,
         xc + dd[:, 2] * s8, yc + dd[:, 3] * s8], axis=1
    ).astype(np.float32)
    mc = bx.max()
    off = (class_ids.astype(np.float32) * (mc + np.float32(1.0))).astype(np.float32)
    b = (bx + off[:, None]).astype(np.float32)
    areas = ((b[:, 2] - b[:, 0]) * (b[:, 3] - b[:, 1])).astype(np.float32)

    xorder = np.argsort(b[:, 0], kind="stable")
    bsx = b[xorder]
    areax = areas[xorder]
    x1s, y1s, x2s, y2s = bsx[:, 0], bsx[:, 1], bsx[:, 2], bsx[:, 3]

    # exact candidate window: for row i all j>i with x1[j] <= x2[i]
    ends = np.searchsorted(x1s, x2s, side="right")
    wneed = max(
        int(ends[t * CH : (t + 1) * CH].max()) - t * CH for t in range(NCHUNK)
    )
    W = max(256, int(np.ceil(wneed / 128.0)) * 128)

    key = (W, stride_f)
    if key not in _PROGRAM_CACHE:
        _PROGRAM_CACHE[key] = _build_program(W, stride_f)
    nc = _PROGRAM_CACHE[key]

    # ---- padded column arrays (pad boxes can never overlap: x1=+huge) ----
    Lc = CPC * CH + W
    PAD = np.float32(3e38)
    L = n + W
    colq = np.empty((5, L), np.float32)
    colq[0, :n] = x1s; colq[0, n:] = PAD
    colq[1, :n] = y1s; colq[1, n:] = PAD
    colq[2, :n] = x2s; colq[2, n:] = PAD
    colq[3, :n] = y2s; colq[3, n:] = PAD
    colq[4, :n] = areax; colq[4, n:] = 0.0

    wbits_row = np.tile(
        np.array([128, 64, 32, 16, 8, 4, 2, 1], np.float32), W // 8
    )
    wbits_in = np.broadcast_to(wbits_row[None, :], (128, W)).copy()

    rowq = np.stack([x1s, y1s, x2s, y2s, areax], axis=1)  # [n, 5]

    in_maps = []
    for c in range(NCORES):
        cols_c = np.ascontiguousarray(
            np.broadcast_to(
                colq[None, :, c * CPC * CH : c * CPC * CH + Lc], (128, 5, Lc)
            )
        )
        rows_c = np.ascontiguousarray(
            rowq[c * CPC * CH : (c + 1) * CPC * CH].reshape(CPC, CH, 5)
            .transpose(1, 0, 2)
        )
        in_maps.append(
            {
                "deltas": deltas,
                "locations": locations,
                "cols": cols_c,
                "rows": rows_c,
                "wbits": wbits_in,
            }
        )

    from concourse import bass_utils

    res = bass_utils.run_bass_kernel_spmd(
        nc, in_maps, core_ids=list(range(NCORES))
    )
    results = res.results

    boxes_out = results[0]["boxes"]

    packed = np.concatenate(
        [results[c]["packed"] for c in range(NCORES)], axis=0
    )  # [NCHUNK, 128, W//8]

    # ---- host: edge extraction + greedy sweep ----
    bits = np.unpackbits(packed, axis=-1).reshape(NCHUNK, CH, W)
    tt, pp, ff = np.nonzero(bits)
    i = tt * CH + pp
    j = tt * CH + ff
    m = (j > i) & (j < n)
    i, j = i[m], j[m]

    sorder = np.argsort(-scores, kind="stable")
    srank = np.empty(n, np.int64)
    srank[sorder] = np.arange(n)
    si = srank[xorder[i]]
    sj = srank[xorder[j]]
    lo = np.minimum(si, sj)
    hi = np.maximum(si, sj)

    keep_s = np.ones(n, bool)
    if len(lo):
        perm = np.argsort(lo, kind="stable")
        lo, hi = lo[perm], hi[perm]
        uniq, start = np.unique(lo, return_index=True)
        start = np.append(start, len(lo))
        for k in range(len(uniq)):
            if keep_s[uniq[k]]:
                keep_s[hi[start[k] : start[k + 1]]] = False
    keep_mask = np.zeros(n, bool)
    keep_mask[sorder] = keep_s

    return boxes_out, keep_mask
